# revision 20
# baseline (speedup 1.0000x reference)
"""Trainium2 Bass kernel for nn_BasicAttentionModel (3-layer GAT + edge MLP).

Strategy (8 NeuronCores, SPMD):
  - Edges partitioned by DESTINATION node range: core c owns dst nodes
    [c*6250, (c+1)*6250). Segment softmax/segment-sum are then core-local.
  - Per GAT layer, each core builds the full per-node feature table
    h_aug = x @ W_aug (+bias) in DRAM (rows [h | a_src | pad]), then streams
    its edges: dma_gather of h_aug[src] rows -> one-hot scatter matmuls on
    the TensorEngine accumulate per-128-dst-node [numerator | denominator]
    windows in PSUM -> per-window softmax normalization + head-mean.
  - Node features are exchanged between layers with an on-chip AllGather.
  - Edge MLP: fp16 x4 table gathered with transpose=True (gives z^T
    directly), 3 matmuls + LeakyRelu/Sigmoid on the scalar engine.
  - BatchNorms, GAT biases and the 1/4 head-mean scale are folded into the
    following layer's weights on the host (parameter-only transforms).
Host-side work is limited to integer index/schedule preprocessing and
parameter folding; all data-dependent compute runs on the NeuronCores.
"""
import sys

sys.path.insert(0, '/opt/trn_rl_repo')

import numpy as np

import concourse.bass as bass
import concourse.bacc as bacc
import concourse.tile as tile
from concourse.tile import add_dep_helper
from concourse import mybir
from concourse.bass_utils import run_bass_kernel_spmd

N = 50000
E = 800000
H = 4
CORES = 8
NPC = N // CORES          # nodes per core (dst range)
HALF = 25000              # gather-table half size (int16 index limit)
WINS = (NPC + 127) // 128  # 49 dst windows per core
LASTW = NPC - (WINS - 1) * 128  # 106 rows in last window
B = 8                     # tiles (of 128 edges) per batch
EPC = E // CORES          # MLP edges per core
NEG_GAT = 0.2
NEG_MLP = 0.12
BN_EPS = 1e-5

F32 = mybir.dt.float32
F16 = mybir.dt.float16
I16 = mybir.dt.int16

# layer configs: cin, C(out per head), HC, table elem (padded row), table dtype
LAYERS = [
    dict(cin=16, C=16, HC=64, elem=128, f16=True),
    dict(cin=16, C=32, HC=128, elem=256, f16=True),
    dict(cin=32, C=64, HC=256, elem=384, f16=True),
]


# ----------------------------------------------------------------------------
# host-side schedule construction (integer index preprocessing only)
# ----------------------------------------------------------------------------

def _wrap_idx(flat):
    """int16 flat [T*128] -> dma_gather wrapped layout [128, T*8]."""
    nb = flat.size // 16
    w16 = flat.reshape(nb, 16).T  # [16, nb]
    return np.tile(w16, (8, 1)).astype(np.int16).copy()


def build_gat_schedule(edge_index):
    src_all = np.concatenate([edge_index[0], np.arange(N, dtype=np.int64)])
    dst_all = np.concatenate([edge_index[1], np.arange(N, dtype=np.int64)])

    # per (core, half-pass, window) edge lists
    per_core = []
    counts = np.zeros((CORES, 2, WINS), np.int64)
    for c in range(CORES):
        sel = (dst_all >= c * NPC) & (dst_all < (c + 1) * NPC)
        s = src_all[sel]
        dl = dst_all[sel] - c * NPC
        hf = (s // HALF).astype(np.int64)
        # order by (half, window, dst) so each (hf, w) run is contiguous
        order = np.lexsort((dl, hf))
        s, dl, hf = s[order], dl[order], hf[order]
        w = dl >> 7
        per_core.append((s % HALF, dl & 127, hf, w))
        for h in range(2):
            m = hf == h
            cw = np.bincount(w[m], minlength=WINS)
            counts[c, h] = cw

    # uniform tile counts per (half, window), each (half, window) padded to
    # whole tiles; each half-pass padded to a multiple of B tiles
    T = np.ceil(counts.max(axis=0) / 128).astype(np.int64)  # [2, WINS]
    raw = []
    for h in range(2):
        tl = [(h, w) for w in range(WINS) for _ in range(int(T[h, w]))]
        while len(tl) % (2 * B) != 0:
            tl.append((h, WINS - 1))
        raw.extend(tl)
    sched = []
    for i, (h, w) in enumerate(raw):
        first = i == 0 or raw[i - 1] != (h, w)
        last = i == len(raw) - 1 or raw[i + 1] != (h, w)
        # eviction mode when this (h, w) run closes
        ev = 'copy' if (h == 0 or T[0, w] == 0) else 'add'
        sched.append((h, w, first, last, ev))
    Ttot = len(sched)
    NB = Ttot // B

    # per-core edge arrays padded into the uniform schedule
    srcidx = np.zeros((CORES, Ttot, 128), np.int16)
    dstloc = np.full((CORES, Ttot, 128), -1.0, np.float32)
    # tile offsets per (hf, w)
    tile_of = {}
    for i, (h, w, first, last, ev) in enumerate(sched):
        tile_of.setdefault((h, w), []).append(i)
    for c in range(CORES):
        s, r, hf, w = per_core[c]
        pos = 0
        for h in range(2):
            m = hf == h
            sh, rh, wh_ = s[m], r[m], w[m]
            st = 0
            for wi in range(WINS):
                n = counts[c, h, wi]
                tls = tile_of.get((h, wi), [])
                ss = sh[st:st + n]
                rr = rh[st:st + n]
                st += n
                for k, t in enumerate(tls):
                    lo = k * 128
                    hi = min(lo + 128, n)
                    if lo >= n:
                        break
                    srcidx[c, t, :hi - lo] = ss[lo:hi]
                    dstloc[c, t, :hi - lo] = rr[lo:hi]
    wrapped = np.stack([_wrap_idx(srcidx[c].reshape(-1)) for c in range(CORES)])
    dstcolT = np.ascontiguousarray(dstloc.transpose(0, 2, 1)).astype(np.float16)  # [C, 128, Ttot]
    dstflat = dstloc.reshape(CORES, NB, B * 128).copy()
    return sched, NB, wrapped, dstcolT, dstflat


def build_mlp_schedule(edge_index):
    s_all = edge_index[0]
    t_all = edge_index[1]
    groups_ct = np.zeros((CORES, 4), np.int64)
    per_core = []
    for c in range(CORES):
        ids = np.arange(c * EPC, (c + 1) * EPC)
        s, t = s_all[ids], t_all[ids]
        g = (s // HALF) * 2 + (t // HALF)
        order = np.argsort(g, kind='stable')
        per_core.append((ids[order], s[order] % HALF, t[order] % HALF))
        groups_ct[c] = np.bincount(g[order], minlength=4)
    Tg = np.ceil(groups_ct.max(axis=0) / 128).astype(np.int64)
    # pad each group to multiple of B tiles so batches are group-homogeneous
    Tg = ((Tg + B - 1) // B) * B
    Tm = int(Tg.sum())
    NBm = Tm // B
    group_of_tile = np.repeat(np.arange(4), Tg)
    goff = np.concatenate([[0], np.cumsum(Tg)])[:4]

    sidx = np.zeros((CORES, Tm * 128), np.int16)
    tidx = np.zeros((CORES, Tm * 128), np.int16)
    outpos = np.full((CORES, Tm * 128), -1, np.int64)
    for c in range(CORES):
        ids, sl, tl = per_core[c]
        st = 0
        for g in range(4):
            n = groups_ct[c, g]
            base = goff[g] * 128
            sidx[c, base:base + n] = sl[st:st + n]
            tidx[c, base:base + n] = tl[st:st + n]
            outpos[c, base:base + n] = ids[st:st + n]
            st += n
    swr = np.stack([_wrap_idx(sidx[c]) for c in range(CORES)])
    twr = np.stack([_wrap_idx(tidx[c]) for c in range(CORES)])
    # batch group (all tiles in batch share a group by construction)
    batch_group = group_of_tile.reshape(NBm, B)[:, 0]
    return NBm, Tm, swr, twr, outpos, batch_group, per_core


def fold_params(params):
    """Fold BN / biases / 0.25 head-mean into per-layer device constants."""
    p = {k: ({kk: np.asarray(vv, np.float64) for kk, vv in v.items()})
         for k, v in params.items() if k != 'mlp'}
    mlp = {k: {kk: np.asarray(vv, np.float64) for kk, vv in v.items()}
           for k, v in params['mlp'].items()}

    def bn_fold(bn):
        s = bn['g'] / np.sqrt(bn['var'] + BN_EPS)
        return s, bn['b'] - bn['mean'] * s

    sx, bx = bn_fold(p['bn_node'])
    se, be = bn_fold(p['bn_edge'])

    out = {}
    prev_bias = None  # bias of previous gat layer (folded forward)
    for li, (nm, cfg) in enumerate(zip(['gat1', 'gat2', 'gat3'], LAYERS)):
        g = p[nm]
        C, HC = cfg['C'], cfg['HC']
        W = g['W']  # [cin, HC]
        if li == 0:
            Wp = sx[:, None] * W
            cp = bx @ W
        else:
            Wp = 0.25 * W
            cp = prev_bias @ W
        A_s = np.zeros((HC, H))
        A_d = np.zeros((HC, H))
        for h in range(H):
            A_s[h * C:(h + 1) * C, h] = g['att_src'][h]
            A_d[h * C:(h + 1) * C, h] = g['att_dst'][h]
        out[f'waug{li}'] = np.concatenate([Wp, Wp @ A_s], 1).astype(np.float32)
        out[f'baug{li}'] = np.concatenate([cp, cp @ A_s]).astype(np.float32)
        out[f'wadst{li}'] = (Wp @ A_d).astype(np.float32)
        out[f'badst{li}'] = (cp @ A_d).astype(np.float32)
        prev_bias = g['bias']

    W1 = mlp['l1']['W']
    out['w1s'] = (0.25 * W1[0:64]).astype(np.float16)
    out['w1t'] = (0.25 * W1[64:128]).astype(np.float16)
    out['w1e'] = (se[:, None] * W1[128:138]).astype(np.float32)
    out['b1m'] = (mlp['l1']['b'] + prev_bias @ W1[0:64] + prev_bias @ W1[64:128]
                  + be @ W1[128:138]).astype(np.float32)
    out['w2m'] = mlp['l2']['W'].astype(np.float32)
    out['b2m'] = mlp['l2']['b'].astype(np.float32)
    out['w3m'] = mlp['l3']['W'].astype(np.float32)
    out['b3m'] = mlp['l3']['b'].astype(np.float32)
    return out


# ----------------------------------------------------------------------------
# device program
# ----------------------------------------------------------------------------

def build_program(sched, NB, NBm, Tm, batch_group):
    import os
    STOP = int(os.environ.get('KERNEL_STOP_AFTER', '99'))
    Ttot = NB * B
    nc = bacc.Bacc("TRN2", debug=False)

    # ---- I/O -----------------------------------------------------------
    xT1 = nc.dram_tensor("xT1", [16, N], F32, kind="ExternalInput")
    xT_own = nc.dram_tensor("xT_own", [16, NPC], F32, kind="ExternalInput")
    wrapped_d = nc.dram_tensor("wrapped", [128, Ttot * 8], I16, kind="ExternalInput")
    dstcolT_d = nc.dram_tensor("dstcolT", [128, Ttot], F16, kind="ExternalInput")
    dstflat_d = nc.dram_tensor("dstflat", [NB, B * 128], F32, kind="ExternalInput")
    swr_d = nc.dram_tensor("swr", [128, Tm * 8], I16, kind="ExternalInput")
    twr_d = nc.dram_tensor("twr", [128, Tm * 8], I16, kind="ExternalInput")
    eT_d = nc.dram_tensor("eT", [10, Tm * 128], F32, kind="ExternalInput")
    iota_rep_d = nc.dram_tensor("iota_rep", [128, B * 128], F16, kind="ExternalInput")
    iota128_d = nc.dram_tensor("iota128", [128, 128], F32, kind="ExternalInput")
    iota_col_d = nc.dram_tensor("iota_col", [128, 1], F32, kind="ExternalInput")
    ones_d = nc.dram_tensor("ones", [1, 128], F32, kind="ExternalInput")
    consts = {}
    for li, cfg in enumerate(LAYERS):
        cin, HC = cfg['cin'], cfg['HC']
        consts[f'waug{li}'] = nc.dram_tensor(f"waug{li}", [cin, HC + 4], F32, kind="ExternalInput")
        consts[f'baug{li}'] = nc.dram_tensor(f"baug{li}", [128, HC + 4], F32, kind="ExternalInput")
        consts[f'wadst{li}'] = nc.dram_tensor(f"wadst{li}", [cin, 4], F32, kind="ExternalInput")
        consts[f'badst{li}'] = nc.dram_tensor(f"badst{li}", [128, 4], F32, kind="ExternalInput")
    w1s_d = nc.dram_tensor("w1s", [64, 64], F16, kind="ExternalInput")
    w1t_d = nc.dram_tensor("w1t", [64, 64], F16, kind="ExternalInput")
    w1e_d = nc.dram_tensor("w1e", [10, 64], F32, kind="ExternalInput")
    b1m_d = nc.dram_tensor("b1m", [64, 1], F32, kind="ExternalInput")
    w2m_d = nc.dram_tensor("w2m", [64, 16], F32, kind="ExternalInput")
    b2m_d = nc.dram_tensor("b2m", [16, 1], F32, kind="ExternalInput")
    w3m_d = nc.dram_tensor("w3m", [16, 1], F32, kind="ExternalInput")
    b3m_d = nc.dram_tensor("b3m", [1, 1], F32, kind="ExternalInput")
    out_d = nc.dram_tensor("out", [Tm // B, B * 128], F32, kind="ExternalOutput")

    with tile.TileContext(nc) as tc:
        with (
            tc.tile_pool(name="const", bufs=1) as cp,
            tc.tile_pool(name="persist", bufs=1) as pp,
            tc.tile_pool(name="dram", bufs=1, space="DRAM") as dp,
        ):
            # long-lived SBUF
            iota_rep = cp.tile([128, B * 128], F16)
            nc.sync.dma_start(iota_rep[:], iota_rep_d[:, :])
            iota128 = cp.tile([128, 128], F32)
            nc.sync.dma_start(iota128[:], iota128_d[:, :])
            iota_col = cp.tile([128, 1], F32)
            nc.sync.dma_start(iota_col[:], iota_col_d[:, :])
            ones_t = cp.tile([1, 128], F32)
            nc.sync.dma_start(ones_t[:], ones_d[:, :])
            wrapped_t = pp.tile([128, Ttot * 8], I16)
            nc.sync.dma_start(wrapped_t[:], wrapped_d[:, :])
            dstcolT_t = pp.tile([128, Ttot], F16)
            nc.sync.dma_start(dstcolT_t[:], dstcolT_d[:, :])

            # DRAM internal tensors
            htabs = []
            for li, cfg in enumerate(LAYERS):
                htabs.append(dp.tile([N, cfg['elem']], F16 if cfg['f16'] else F32, name=f'htab{li}'))
            cin1 = dp.tile([16, NPC], F32)
            cout1 = dp.tile([16 * CORES, NPC], F32, addr_space="Shared")
            cin2 = dp.tile([32, NPC], F32)
            cout2 = dp.tile([32 * CORES, NPC], F32, addr_space="Shared")
            cin3 = dp.tile([NPC, 128], F16)
            cout3 = dp.tile([N, 128], F16, addr_space="Shared")

            x4_local = pp.tile([128, WINS, 128], F16)

            for li, cfg in enumerate(LAYERS):
                if li >= STOP:
                    break
                cin, C, HC, elem = cfg['cin'], cfg['C'], cfg['HC'], cfg['elem']
                tdt = F16 if cfg['f16'] else F32
                htab = htabs[li]
                htab_writes = [[], []]
                with (
                    tc.tile_pool(name=f"hb{li}", bufs=3) as hb,
                    tc.tile_pool(name=f"hbp{li}", bufs=3, space="PSUM") as hbp,
                ):
                    waug_t = hb.tile([cin, HC + 4], F32, tag="w")
                    nc.sync.dma_start(waug_t[:], consts[f'waug{li}'][:, :])
                    baug_t = hb.tile([128, HC + 4], F32, tag="b")
                    nc.sync.dma_start(baug_t[:], consts[f'baug{li}'][:, :])
                    # h table build: h_aug = x @ Waug + baug
                    if li == 0:
                        blocks = [(xT1, 0, N)]
                    elif li == 1:
                        blocks = [(cout1, c * 16, NPC) for c in range(CORES)]
                    else:
                        blocks = [(cout2, c * 32, NPC) for c in range(CORES)]
                    row0 = 0
                    for (srcT, prow, ncols) in blocks:
                        for j0 in range(0, ncols, 128):
                            j1 = min(j0 + 128, ncols)
                            nn = j1 - j0
                            lhs = hb.tile([cin, 128], F32, tag="lhs")
                            nc.sync.dma_start(lhs[:, :nn], srcT[prow:prow + cin, j0:j1])
                            hp = hbp.tile([128, HC + 4], F32, space="PSUM", tag="hp")
                            nc.tensor.matmul(out=hp[:nn, :], lhsT=lhs[:, :nn],
                                             rhs=waug_t[:], start=True, stop=True)
                            hs = hb.tile([128, HC + 4], tdt, tag="hs")
                            nc.vector.tensor_tensor(out=hs[:nn, :], in0=hp[:nn, :],
                                                    in1=baug_t[:nn, :],
                                                    op=mybir.AluOpType.add)
                            wi_ = nc.scalar.dma_start(htab[row0:row0 + nn, 0:HC + 4], hs[:nn, :])
                            if row0 < HALF:
                                htab_writes[0].append(wi_)
                            if row0 + nn > HALF:
                                htab_writes[1].append(wi_)
                            row0 += nn

                # a_dst for own dst range
                xT_src = [xT_own, cin1, cin2][li]
                with (
                    tc.tile_pool(name=f"ad{li}", bufs=2) as ad,
                    tc.tile_pool(name=f"adp{li}", bufs=2, space="PSUM") as adp,
                ):
                    wadst_t = ad.tile([cin, 4], F32, tag="w")
                    nc.sync.dma_start(wadst_t[:], consts[f'wadst{li}'][:, :])
                    badst_t = ad.tile([128, 4], F32, tag="b")
                    nc.sync.dma_start(badst_t[:], consts[f'badst{li}'][:, :])
                    adst_all = pp.tile([128, WINS, 4], F32, tag="adst")
                    for w in range(WINS):
                        j0 = w * 128
                        j1 = min(j0 + 128, NPC)
                        nn = j1 - j0
                        xad = ad.tile([cin, 128], F32, tag="xad")
                        nc.scalar.dma_start(xad[:, :nn], xT_src[:, j0:j1])
                        ap_ = adp.tile([128, 4], F32, space="PSUM", tag="ap")
                        nc.tensor.matmul(out=ap_[:nn, :], lhsT=xad[:, :nn],
                                         rhs=wadst_t[:], start=True, stop=True)
                        nc.vector.tensor_tensor(out=adst_all[:nn, w, :], in0=ap_[:nn, :],
                                                in1=badst_t[:nn, :],
                                                op=mybir.AluOpType.add)

                # edge phase
                acc = pp.tile([128, WINS, HC + 4], F32, tag="acc")
                with (
                    tc.tile_pool(name=f"eg{li}", bufs=3) as eg,
                    tc.tile_pool(name=f"egp{li}", bufs=2, space="PSUM") as egp,
                    tc.tile_pool(name=f"winp{li}", bufs=2, space="PSUM") as winp,
                ):
                    psum_w = None
                    for bi in range(NB):
                        dflat = eg.tile([1, B * 128], F32, tag="dflat")
                        nc.sync.dma_start(dflat[:], dstflat_d[bi:bi + 1, :])
                        prow = egp.tile([128, B * 128], F32, space="PSUM", tag="prow")
                        for hh in range(B * 128 // 512):
                            nc.tensor.matmul(out=prow[:, hh * 512:(hh + 1) * 512],
                                             lhsT=ones_t[:],
                                             rhs=dflat[:, hh * 512:(hh + 1) * 512],
                                             start=True, stop=True)
                        oht = eg.tile([128, B * 128], F32, tag="oht")
                        nc.vector.tensor_scalar(
                            out=oht[:], in0=prow[:], scalar1=iota_col[:],
                            scalar2=None, op0=mybir.AluOpType.is_equal)
                        oh = eg.tile([128, B * 128], tdt, tag="oh")
                        nc.vector.tensor_tensor(
                            out=oh[:], in0=iota_rep[:],
                            in1=dstcolT_t[:, bi * B:(bi + 1) * B]
                                .unsqueeze(-1).to_broadcast([128, B, 128]),
                            op=mybir.AluOpType.is_equal)
                        if bi % 2 == 0:
                            g2 = eg.tile([128, 2 * B, elem], tdt, tag="g")
                            gi_ = nc.gpsimd.dma_gather(
                                g2[:], htab[0:HALF, :] if sched[bi * B][0] == 0
                                else htab[HALF:N, :],
                                wrapped_t[:, bi * B * 8:(bi + 2) * B * 8],
                                2 * B * 128, 2 * B * 128, elem,
                                single_packet=False)
                            for wi_ in htab_writes[sched[bi * B][0]]:
                                add_dep_helper(gi_.ins, wi_.ins, sync=True,
                                               reason="gather reads h table")
                        g = g2[:, (bi % 2) * B:(bi % 2) * B + B, :]
                        adst_e = egp.tile([128, B, 4], F32, space="PSUM", tag="adst_e")
                        for t in range(B):
                            hf, w, first, last, ev = sched[bi * B + t]
                            nc.tensor.matmul(
                                out=adst_e[:, t, :],
                                lhsT=oht[:, t * 128:(t + 1) * 128],
                                rhs=adst_all[:, w, :], start=True, stop=True)
                        lg = eg.tile([128, B, 4], F32, tag="lg")
                        nc.vector.tensor_tensor(out=lg[:], in0=g[:, :, HC:HC + 4],
                                                in1=adst_e[:],
                                                op=mybir.AluOpType.add)
                        lr = eg.tile([128, B, 4], F32, tag="lr")
                        nc.vector.scalar_tensor_tensor(
                            out=lr[:], in0=lg[:], scalar=NEG_GAT, in1=lg[:],
                            op0=mybir.AluOpType.mult, op1=mybir.AluOpType.max)
                        wt = eg.tile([128, B, 4], tdt, tag="wt")
                        nc.scalar.activation(wt[:], lr[:],
                                             mybir.ActivationFunctionType.Exp)
                        rhs = eg.tile([128, B, HC + 4], tdt, tag="rhs")
                        nc.vector.tensor_tensor(
                            out=rhs[:, :, 0:HC].rearrange("p b (h c) -> p b h c", h=H),
                            in0=g[:, :, 0:HC].rearrange("p b (h c) -> p b h c", h=H),
                            in1=wt[:].unsqueeze(-1).to_broadcast([128, B, 4, C]),
                            op=mybir.AluOpType.mult)
                        nc.vector.tensor_copy(out=rhs[:, :, HC:HC + 4], in_=wt[:])
                        for t in range(B):
                            hf, w, first, last, ev = sched[bi * B + t]
                            if first:
                                psum_w = winp.tile([128, HC + 4], F32, space="PSUM",
                                                   tag="win")
                            nc.tensor.matmul(out=psum_w[:],
                                             lhsT=oh[:, t * 128:(t + 1) * 128],
                                             rhs=rhs[:, t, :],
                                             start=first, stop=last)
                            if last:
                                if ev == 'copy':
                                    nc.vector.tensor_copy(out=acc[:, w, :], in_=psum_w[:])
                                else:
                                    nc.vector.tensor_tensor(
                                        out=acc[:, w, :], in0=acc[:, w, :],
                                        in1=psum_w[:], op=mybir.AluOpType.add)

                # normalization + output
                with (
                    tc.tile_pool(name=f"no{li}", bufs=3) as no,
                    tc.tile_pool(name=f"nop{li}", bufs=2, space="PSUM") as nop,
                ):
                    cin_t = [cin1, cin2, None][li]
                    if li < 2:
                        ident = no.tile([128, 128], F32, tag="id")
                        # identity built from iota compare: ident[p,j] = (iota_row[j]==p)
                        nc.vector.tensor_scalar(
                            out=ident[:], in0=iota128[:], scalar1=iota_col[:],
                            scalar2=None, op0=mybir.AluOpType.is_equal)
                    for w in range(WINS):
                        nn = 128 if w < WINS - 1 else LASTW
                        rec = no.tile([128, 4], F32, tag="rec")
                        nc.vector.reciprocal(rec[:nn, :], acc[:nn, w, HC:HC + 4])
                        tmp = no.tile([128, HC], F32, tag="tmp")
                        nc.vector.tensor_tensor(
                            out=tmp[:nn, :].rearrange("p (h c) -> p h c", h=H),
                            in0=acc[:nn, w, 0:HC].rearrange("p (h c) -> p h c", h=H),
                            in1=rec[:nn, :].unsqueeze(-1).to_broadcast([nn, 4, C]),
                            op=mybir.AluOpType.mult)
                        red = no.tile([128, C], F32, tag="red")
                        nc.vector.tensor_reduce(
                            out=red[:nn, :],
                            in_=tmp[:nn, :].rearrange("p (h c) -> p c h", h=H),
                            axis=mybir.AxisListType.X, op=mybir.AluOpType.add)
                        if li < 2:
                            tp = nop.tile([C, 128], F32, space="PSUM", tag="tp")
                            nc.tensor.transpose(out=tp[:], in_=red[:], identity=ident[:])
                            tps = no.tile([C, 128], F32, tag="tps")
                            nc.vector.tensor_copy(out=tps[:, :nn], in_=tp[:, :nn])
                            nc.sync.dma_start(cin_t[:, w * 128:w * 128 + nn],
                                              tps[:, :nn])
                        else:
                            nc.vector.tensor_copy(out=x4_local[:nn, w, 0:64],
                                                  in_=red[:nn, :])

                # inter-layer collective
                if li == 0:
                    nc.gpsimd.collective_compute(
                        "AllGather", mybir.AluOpType.bypass,
                        replica_groups=[list(range(CORES))],
                        ins=[cin1[:].opt()], outs=[cout1[:].opt()])
                elif li == 1:
                    nc.gpsimd.collective_compute(
                        "AllGather", mybir.AluOpType.bypass,
                        replica_groups=[list(range(CORES))],
                        ins=[cin2[:].opt()], outs=[cout2[:].opt()])
                else:
                    for w in range(WINS):
                        nn = 128 if w < WINS - 1 else LASTW
                        nc.sync.dma_start(cin3[w * 128:w * 128 + nn, :],
                                          x4_local[:nn, w, :])
                    cc3 = nc.gpsimd.collective_compute(
                        "AllGather", mybir.AluOpType.bypass,
                        replica_groups=[list(range(CORES))],
                        ins=[cin3[:].opt()], outs=[cout3[:].opt()])

            # ---------------- MLP phase ----------------
            with (
                tc.tile_pool(name="mc", bufs=1) as mc,
                tc.tile_pool(name="mw", bufs=3) as mw,
                tc.tile_pool(name="mp", bufs=2, space="PSUM") as mp,
            ):
                w1s_t = mc.tile([64, 64], F16)
                nc.sync.dma_start(w1s_t[:], w1s_d[:, :])
                w1t_t = mc.tile([64, 64], F16)
                nc.sync.dma_start(w1t_t[:], w1t_d[:, :])
                w1e_t = mc.tile([10, 64], F32)
                nc.sync.dma_start(w1e_t[:], w1e_d[:, :])
                b1m_t = mc.tile([64, 1], F32)
                nc.sync.dma_start(b1m_t[:], b1m_d[:, :])
                w2m_t = mc.tile([64, 16], F32)
                nc.sync.dma_start(w2m_t[:], w2m_d[:, :])
                b2m_t = mc.tile([16, 1], F32)
                nc.sync.dma_start(b2m_t[:], b2m_d[:, :])
                w3m_t = mc.tile([16, 1], F32)
                nc.sync.dma_start(w3m_t[:], w3m_d[:, :])
                b3m_t = mc.tile([1, 1], F32)
                nc.sync.dma_start(b3m_t[:], b3m_d[:, :])
                swr_t = mc.tile([128, Tm * 8], I16, tag="swr")
                nc.sync.dma_start(swr_t[:], swr_d[:, :])
                twr_t = mc.tile([128, Tm * 8], I16, tag="twr")
                nc.sync.dma_start(twr_t[:], twr_d[:, :])

                NE = B * 128
                for bm in range(min(NBm if STOP >= 4 else 0, int(os.environ.get('KERNEL_MLP_BATCHES', '100000')))):
                    gidx = int(batch_group[bm])
                    sh, th = gidx // 2, gidx % 2
                    gs = mw.tile([128, 1, NE], F16, tag="gs")
                    gt = mw.tile([128, 1, NE], F16, tag="gt")
                    gsi = nc.gpsimd.dma_gather(
                        gs[:], cout3[sh * HALF:(sh + 1) * HALF, :],
                        swr_t[:, bm * B * 8:(bm + 1) * B * 8],
                        NE, NE, 128, transpose=True, single_packet=False)
                    add_dep_helper(gsi.ins, cc3.ins, sync=True,
                                   reason="mlp gather reads x4 allgather")
                    gti = nc.gpsimd.dma_gather(
                        gt[:], cout3[th * HALF:(th + 1) * HALF, :],
                        twr_t[:, bm * B * 8:(bm + 1) * B * 8],
                        NE, NE, 128, transpose=True, single_packet=False)
                    add_dep_helper(gti.ins, cc3.ins, sync=True,
                                   reason="mlp gather reads x4 allgather")
                    et = mw.tile([10, NE], F32, tag="et")
                    nc.sync.dma_start(et[:], eT_d[:, bm * NE:(bm + 1) * NE])
                    res = mw.tile([1, NE], F32, tag="res")
                    for hh in range(NE // 512):
                        sl = slice(hh * 512, (hh + 1) * 512)
                        o1 = mp.tile([64, 512], F32, space="PSUM", tag="o1")
                        nc.tensor.matmul(out=o1[:], lhsT=w1s_t[:],
                                         rhs=gs[0:64, 0, sl], start=True, stop=False)
                        nc.tensor.matmul(out=o1[:], lhsT=w1t_t[:],
                                         rhs=gt[0:64, 0, sl], start=False, stop=False)
                        nc.tensor.matmul(out=o1[:], lhsT=w1e_t[:],
                                         rhs=et[:, sl], start=False, stop=True)
                        o1b = mw.tile([64, 512], F32, tag="o1b")
                        nc.vector.tensor_scalar(
                            out=o1b[:], in0=o1[:], scalar1=b1m_t[:, 0:1],
                            scalar2=None, op0=mybir.AluOpType.add)
                        o1s = mw.tile([64, 512], F32, tag="o1s")
                        nc.vector.scalar_tensor_tensor(
                            out=o1s[:], in0=o1b[:], scalar=NEG_MLP, in1=o1b[:],
                            op0=mybir.AluOpType.mult, op1=mybir.AluOpType.max)
                        o2 = mp.tile([16, 512], F32, space="PSUM", tag="o2")
                        nc.tensor.matmul(out=o2[:], lhsT=w2m_t[:], rhs=o1s[:],
                                         start=True, stop=True)
                        o2b = mw.tile([16, 512], F32, tag="o2b")
                        nc.vector.tensor_scalar(
                            out=o2b[:], in0=o2[:], scalar1=b2m_t[:, 0:1],
                            scalar2=None, op0=mybir.AluOpType.add)
                        o2s = mw.tile([16, 512], F32, tag="o2s")
                        nc.vector.scalar_tensor_tensor(
                            out=o2s[:], in0=o2b[:], scalar=NEG_MLP, in1=o2b[:],
                            op0=mybir.AluOpType.mult, op1=mybir.AluOpType.max)
                        o3 = mp.tile([1, 512], F32, space="PSUM", tag="o3")
                        nc.tensor.matmul(out=o3[:], lhsT=w3m_t[:], rhs=o2s[:],
                                         start=True, stop=True)
                        nc.scalar.activation(res[:, sl], o3[:],
                                             mybir.ActivationFunctionType.Sigmoid,
                                             bias=b3m_t[:, 0:1], scale=1.0)
                    nc.sync.dma_start(out_d[bm:bm + 1, :], res[:])

    nc.compile()
    return nc


# ----------------------------------------------------------------------------
# entry point
# ----------------------------------------------------------------------------

_CACHE = {}
LAST_EXEC_NS = None
LAST_RUN_S = None


def kernel(x, e, edge_index, params):
    x = np.asarray(x, np.float32)
    e = np.asarray(e, np.float32)
    ei = np.asarray(edge_index).astype(np.int64)

    sched, NB, wrapped, dstcolT, dstflat = build_gat_schedule(ei)
    NBm, Tm, swr, twr, outpos, batch_group, mlp_pc = build_mlp_schedule(ei)
    folded = fold_params(params)

    key = (NB, NBm, tuple(s[0] * 1000 + s[1] for s in sched[::37]),
           tuple(batch_group.tolist()))
    if key not in _CACHE:
        _CACHE[key] = build_program(sched, NB, NBm, Tm, batch_group)
    nc = _CACHE[key]

    xT1 = np.ascontiguousarray(x.T)
    iota_rep = np.tile(np.arange(128, dtype=np.float16)[None, :], (128, B))
    iota128 = np.tile(np.arange(128, dtype=np.float32)[None, :], (128, 1))
    iota_col = np.arange(128, dtype=np.float32)[:, None].copy()
    ones = np.ones((1, 128), np.float32)

    in_maps = []
    for c in range(CORES):
        ids, _, _ = mlp_pc[c]
        eT_c = np.zeros((10, Tm * 128), np.float32)
        valid = outpos[c] >= 0
        eT_c[:, valid] = e[outpos[c][valid]].T
        m = {
            "xT1": xT1,
            "xT_own": np.ascontiguousarray(xT1[:, c * NPC:(c + 1) * NPC]),
            "wrapped": wrapped[c],
            "dstcolT": dstcolT[c],
            "dstflat": dstflat[c],
            "swr": swr[c], "twr": twr[c], "eT": eT_c,
            "iota_rep": iota_rep, "iota128": iota128,
            "iota_col": iota_col, "ones": ones,
        }
        for li in range(3):
            m[f"waug{li}"] = folded[f'waug{li}']
            m[f"baug{li}"] = np.tile(folded[f'baug{li}'][None, :], (128, 1)).copy()
            m[f"wadst{li}"] = folded[f'wadst{li}']
            m[f"badst{li}"] = np.tile(folded[f'badst{li}'][None, :], (128, 1)).copy()
        m.update({"w1s": folded['w1s'], "w1t": folded['w1t'], "w1e": folded['w1e'],
                  "b1m": folded['b1m'][:, None].copy(),
                  "w2m": folded['w2m'], "b2m": folded['b2m'][:, None].copy(),
                  "w3m": folded['w3m'], "b3m": folded['b3m'][:, None].copy()})
        in_maps.append(m)

    import os as _os, time as _time
    trace = _os.environ.get('KERNEL_TRACE', '0') == '1'
    global LAST_EXEC_NS, LAST_RUN_S
    t0 = _time.time()
    try:
        res = run_bass_kernel_spmd(nc, in_maps, core_ids=list(range(CORES)),
                                   trace=trace)
    except Exception:
        if not trace:
            raise
        res = run_bass_kernel_spmd(nc, in_maps, core_ids=list(range(CORES)))
    LAST_RUN_S = _time.time() - t0
    LAST_EXEC_NS = getattr(res, 'exec_time_ns', None)

    out = np.zeros((E,), np.float32)
    for c in range(CORES):
        flat = res.results[c]["out"].reshape(-1)
        valid = outpos[c] >= 0
        out[outpos[c][valid]] = flat[valid]
    return out[:, None].astype(np.float32)


if __name__ == "__main__":
    data = np.load('/root/problem/cache_inputs.npz')
    import pickle
    with open('/root/problem/cache_params.pkl', 'rb') as f:
        params = pickle.load(f)
    out = kernel(data['x'], data['e'], data['edge_index'], params)
    np.save('/root/problem/kernel_out.npy', out)
    print("kernel out", out.shape, out[:5, 0])


# revision 21
# speedup vs baseline: 1.0669x; 1.0669x over previous
"""Trainium2 Bass kernel for nn_BasicAttentionModel (3-layer GAT + edge MLP).

Strategy (8 NeuronCores, SPMD):
  - Edges partitioned by DESTINATION node range: core c owns dst nodes
    [c*6250, (c+1)*6250). Segment softmax/segment-sum are then core-local.
  - Per GAT layer, each core builds the full per-node feature table
    h_aug = x @ W_aug (+bias) in DRAM (rows [h | a_src | pad]), then streams
    its edges: dma_gather of h_aug[src] rows -> one-hot scatter matmuls on
    the TensorEngine accumulate per-128-dst-node [numerator | denominator]
    windows in PSUM -> per-window softmax normalization + head-mean.
  - Node features are exchanged between layers with an on-chip AllGather.
  - Edge MLP: fp16 x4 table gathered with transpose=True (gives z^T
    directly), 3 matmuls + LeakyRelu/Sigmoid on the scalar engine.
  - BatchNorms, GAT biases and the 1/4 head-mean scale are folded into the
    following layer's weights on the host (parameter-only transforms).
Host-side work is limited to integer index/schedule preprocessing and
parameter folding; all data-dependent compute runs on the NeuronCores.
"""
import sys

sys.path.insert(0, '/opt/trn_rl_repo')

import numpy as np

import concourse.bass as bass
import concourse.bacc as bacc
import concourse.tile as tile
from concourse.tile import add_dep_helper
from concourse import mybir
from concourse.bass_utils import run_bass_kernel_spmd

N = 50000
E = 800000
H = 4
CORES = 8
NPC = N // CORES          # nodes per core (dst range)
HALF = 25000              # gather-table half size (int16 index limit)
WINS = (NPC + 127) // 128  # 49 dst windows per core
LASTW = NPC - (WINS - 1) * 128  # 106 rows in last window
B = 8                     # tiles (of 128 edges) per batch
EPC = E // CORES          # MLP edges per core
NEG_GAT = 0.2
NEG_MLP = 0.12
BN_EPS = 1e-5

F32 = mybir.dt.float32
F16 = mybir.dt.float16
I16 = mybir.dt.int16

# layer configs: cin, C(out per head), HC, table elem (padded row), table dtype
LAYERS = [
    dict(cin=16, C=16, HC=64, elem=128, f16=True),
    dict(cin=16, C=32, HC=128, elem=256, f16=True),
    dict(cin=32, C=64, HC=256, elem=384, f16=True),
]


# ----------------------------------------------------------------------------
# host-side schedule construction (integer index preprocessing only)
# ----------------------------------------------------------------------------

def _wrap_idx(flat):
    """int16 flat [T*128] -> dma_gather wrapped layout [128, T*8]."""
    nb = flat.size // 16
    w16 = flat.reshape(nb, 16).T  # [16, nb]
    return np.tile(w16, (8, 1)).astype(np.int16).copy()


def build_gat_schedule(edge_index):
    src_all = np.concatenate([edge_index[0], np.arange(N, dtype=np.int64)])
    dst_all = np.concatenate([edge_index[1], np.arange(N, dtype=np.int64)])

    # per (core, half-pass, window) edge lists
    per_core = []
    counts = np.zeros((CORES, 2, WINS), np.int64)
    for c in range(CORES):
        sel = (dst_all >= c * NPC) & (dst_all < (c + 1) * NPC)
        s = src_all[sel]
        dl = dst_all[sel] - c * NPC
        hf = (s // HALF).astype(np.int64)
        # order by (half, window, dst) so each (hf, w) run is contiguous
        order = np.lexsort((dl, hf))
        s, dl, hf = s[order], dl[order], hf[order]
        w = dl >> 7
        per_core.append((s % HALF, dl & 127, hf, w))
        for h in range(2):
            m = hf == h
            cw = np.bincount(w[m], minlength=WINS)
            counts[c, h] = cw

    # uniform tile counts per (half, window), each (half, window) padded to
    # whole tiles; each half-pass padded to a multiple of B tiles
    T = np.ceil(counts.max(axis=0) / 128).astype(np.int64)  # [2, WINS]
    raw = []
    for h in range(2):
        tl = [(h, w) for w in range(WINS) for _ in range(int(T[h, w]))]
        while len(tl) % (2 * B) != 0:
            tl.append((h, WINS - 1))
        raw.extend(tl)
    sched = []
    for i, (h, w) in enumerate(raw):
        first = i == 0 or raw[i - 1] != (h, w)
        last = i == len(raw) - 1 or raw[i + 1] != (h, w)
        # eviction mode when this (h, w) run closes
        ev = 'copy' if (h == 0 or T[0, w] == 0) else 'add'
        sched.append((h, w, first, last, ev))
    Ttot = len(sched)
    NB = Ttot // B

    # per-core edge arrays padded into the uniform schedule
    srcidx = np.zeros((CORES, Ttot, 128), np.int16)
    dstloc = np.full((CORES, Ttot, 128), -1.0, np.float32)
    # tile offsets per (hf, w)
    tile_of = {}
    for i, (h, w, first, last, ev) in enumerate(sched):
        tile_of.setdefault((h, w), []).append(i)
    for c in range(CORES):
        s, r, hf, w = per_core[c]
        pos = 0
        for h in range(2):
            m = hf == h
            sh, rh, wh_ = s[m], r[m], w[m]
            st = 0
            for wi in range(WINS):
                n = counts[c, h, wi]
                tls = tile_of.get((h, wi), [])
                ss = sh[st:st + n]
                rr = rh[st:st + n]
                st += n
                for k, t in enumerate(tls):
                    lo = k * 128
                    hi = min(lo + 128, n)
                    if lo >= n:
                        break
                    srcidx[c, t, :hi - lo] = ss[lo:hi]
                    dstloc[c, t, :hi - lo] = rr[lo:hi]
    wrapped = np.stack([_wrap_idx(srcidx[c].reshape(-1)) for c in range(CORES)])
    dstcolT = np.ascontiguousarray(dstloc.transpose(0, 2, 1)).astype(np.float16)  # [C, 128, Ttot]
    dstflat = dstloc.reshape(CORES, NB, B * 128).copy()
    return sched, NB, wrapped, dstcolT, dstflat


def build_mlp_schedule(edge_index):
    s_all = edge_index[0]
    t_all = edge_index[1]
    groups_ct = np.zeros((CORES, 4), np.int64)
    per_core = []
    for c in range(CORES):
        ids = np.arange(c * EPC, (c + 1) * EPC)
        s, t = s_all[ids], t_all[ids]
        g = (s // HALF) * 2 + (t // HALF)
        order = np.argsort(g, kind='stable')
        per_core.append((ids[order], s[order] % HALF, t[order] % HALF))
        groups_ct[c] = np.bincount(g[order], minlength=4)
    Tg = np.ceil(groups_ct.max(axis=0) / 128).astype(np.int64)
    # pad each group to multiple of B tiles so batches are group-homogeneous
    Tg = ((Tg + B - 1) // B) * B
    Tm = int(Tg.sum())
    NBm = Tm // B
    group_of_tile = np.repeat(np.arange(4), Tg)
    goff = np.concatenate([[0], np.cumsum(Tg)])[:4]

    sidx = np.zeros((CORES, Tm * 128), np.int16)
    tidx = np.zeros((CORES, Tm * 128), np.int16)
    outpos = np.full((CORES, Tm * 128), -1, np.int64)
    for c in range(CORES):
        ids, sl, tl = per_core[c]
        st = 0
        for g in range(4):
            n = groups_ct[c, g]
            base = goff[g] * 128
            sidx[c, base:base + n] = sl[st:st + n]
            tidx[c, base:base + n] = tl[st:st + n]
            outpos[c, base:base + n] = ids[st:st + n]
            st += n
    swr = np.stack([_wrap_idx(sidx[c]) for c in range(CORES)])
    twr = np.stack([_wrap_idx(tidx[c]) for c in range(CORES)])
    # batch group (all tiles in batch share a group by construction)
    batch_group = group_of_tile.reshape(NBm, B)[:, 0]
    return NBm, Tm, swr, twr, outpos, batch_group, per_core


def fold_params(params):
    """Fold BN / biases / 0.25 head-mean into per-layer device constants."""
    p = {k: ({kk: np.asarray(vv, np.float64) for kk, vv in v.items()})
         for k, v in params.items() if k != 'mlp'}
    mlp = {k: {kk: np.asarray(vv, np.float64) for kk, vv in v.items()}
           for k, v in params['mlp'].items()}

    def bn_fold(bn):
        s = bn['g'] / np.sqrt(bn['var'] + BN_EPS)
        return s, bn['b'] - bn['mean'] * s

    sx, bx = bn_fold(p['bn_node'])
    se, be = bn_fold(p['bn_edge'])

    out = {}
    prev_bias = None  # bias of previous gat layer (folded forward)
    for li, (nm, cfg) in enumerate(zip(['gat1', 'gat2', 'gat3'], LAYERS)):
        g = p[nm]
        C, HC = cfg['C'], cfg['HC']
        W = g['W']  # [cin, HC]
        if li == 0:
            Wp = sx[:, None] * W
            cp = bx @ W
        else:
            Wp = 0.25 * W
            cp = prev_bias @ W
        A_s = np.zeros((HC, H))
        A_d = np.zeros((HC, H))
        for h in range(H):
            A_s[h * C:(h + 1) * C, h] = g['att_src'][h]
            A_d[h * C:(h + 1) * C, h] = g['att_dst'][h]
        out[f'waug{li}'] = np.concatenate([Wp, Wp @ A_s], 1).astype(np.float32)
        out[f'baug{li}'] = np.concatenate([cp, cp @ A_s]).astype(np.float32)
        out[f'wadst{li}'] = (Wp @ A_d).astype(np.float32)
        out[f'badst{li}'] = (cp @ A_d).astype(np.float32)
        prev_bias = g['bias']

    W1 = mlp['l1']['W']
    out['w1s'] = (0.25 * W1[0:64]).astype(np.float16)
    out['w1t'] = (0.25 * W1[64:128]).astype(np.float16)
    out['w1e'] = (se[:, None] * W1[128:138]).astype(np.float32)
    out['b1m'] = (mlp['l1']['b'] + prev_bias @ W1[0:64] + prev_bias @ W1[64:128]
                  + be @ W1[128:138]).astype(np.float32)
    out['w2m'] = mlp['l2']['W'].astype(np.float32)
    out['b2m'] = mlp['l2']['b'].astype(np.float32)
    out['w3m'] = mlp['l3']['W'].astype(np.float32)
    out['b3m'] = mlp['l3']['b'].astype(np.float32)
    return out


# ----------------------------------------------------------------------------
# device program
# ----------------------------------------------------------------------------

def build_program(sched, NB, NBm, Tm, batch_group):
    import os
    STOP = int(os.environ.get('KERNEL_STOP_AFTER', '99'))
    Ttot = NB * B
    nc = bacc.Bacc("TRN2", debug=False)

    # ---- I/O -----------------------------------------------------------
    xT1 = nc.dram_tensor("xT1", [16, N], F32, kind="ExternalInput")
    xT_own = nc.dram_tensor("xT_own", [16, NPC], F32, kind="ExternalInput")
    wrapped_d = nc.dram_tensor("wrapped", [128, Ttot * 8], I16, kind="ExternalInput")
    dstcolT_d = nc.dram_tensor("dstcolT", [128, Ttot], F16, kind="ExternalInput")
    dstflat_d = nc.dram_tensor("dstflat", [NB, B * 128], F32, kind="ExternalInput")
    swr_d = nc.dram_tensor("swr", [128, Tm * 8], I16, kind="ExternalInput")
    twr_d = nc.dram_tensor("twr", [128, Tm * 8], I16, kind="ExternalInput")
    eT_d = nc.dram_tensor("eT", [10, Tm * 128], F32, kind="ExternalInput")
    iota_rep_d = nc.dram_tensor("iota_rep", [128, B * 128], F16, kind="ExternalInput")
    iota128_d = nc.dram_tensor("iota128", [128, 128], F32, kind="ExternalInput")
    iota_col_d = nc.dram_tensor("iota_col", [128, 1], F32, kind="ExternalInput")
    ones_d = nc.dram_tensor("ones", [1, 128], F32, kind="ExternalInput")
    consts = {}
    for li, cfg in enumerate(LAYERS):
        cin, HC = cfg['cin'], cfg['HC']
        consts[f'waug{li}'] = nc.dram_tensor(f"waug{li}", [cin, HC + 4], F32, kind="ExternalInput")
        consts[f'baug{li}'] = nc.dram_tensor(f"baug{li}", [128, HC + 4], F32, kind="ExternalInput")
        consts[f'wadst{li}'] = nc.dram_tensor(f"wadst{li}", [cin, 4], F32, kind="ExternalInput")
        consts[f'badst{li}'] = nc.dram_tensor(f"badst{li}", [128, 4], F32, kind="ExternalInput")
    w1s_d = nc.dram_tensor("w1s", [64, 64], F16, kind="ExternalInput")
    w1t_d = nc.dram_tensor("w1t", [64, 64], F16, kind="ExternalInput")
    w1e_d = nc.dram_tensor("w1e", [10, 64], F32, kind="ExternalInput")
    b1m_d = nc.dram_tensor("b1m", [64, 1], F32, kind="ExternalInput")
    w2m_d = nc.dram_tensor("w2m", [64, 16], F32, kind="ExternalInput")
    b2m_d = nc.dram_tensor("b2m", [16, 1], F32, kind="ExternalInput")
    w3m_d = nc.dram_tensor("w3m", [16, 1], F32, kind="ExternalInput")
    b3m_d = nc.dram_tensor("b3m", [1, 1], F32, kind="ExternalInput")
    out_d = nc.dram_tensor("out", [Tm // B, B * 128], F32, kind="ExternalOutput")

    with tile.TileContext(nc) as tc:
        with (
            tc.tile_pool(name="const", bufs=1) as cp,
            tc.tile_pool(name="persist", bufs=1) as pp,
            tc.tile_pool(name="dram", bufs=1, space="DRAM") as dp,
        ):
            # long-lived SBUF
            iota_rep = cp.tile([128, B * 128], F16)
            nc.sync.dma_start(iota_rep[:], iota_rep_d[:, :])
            iota128 = cp.tile([128, 128], F32)
            nc.sync.dma_start(iota128[:], iota128_d[:, :])
            iota_col = cp.tile([128, 1], F32)
            nc.sync.dma_start(iota_col[:], iota_col_d[:, :])
            ones_t = cp.tile([1, 128], F32)
            nc.sync.dma_start(ones_t[:], ones_d[:, :])
            wrapped_t = pp.tile([128, Ttot * 8], I16)
            nc.sync.dma_start(wrapped_t[:], wrapped_d[:, :])
            dstcolT_t = pp.tile([128, Ttot], F16)
            nc.sync.dma_start(dstcolT_t[:], dstcolT_d[:, :])

            # DRAM internal tensors
            htabs = []
            for li, cfg in enumerate(LAYERS):
                htabs.append(dp.tile([N, cfg['elem']], F16 if cfg['f16'] else F32, name=f'htab{li}'))
            cin1 = dp.tile([16, NPC], F32)
            cout1 = dp.tile([16 * CORES, NPC], F32, addr_space="Shared")
            cin2 = dp.tile([32, NPC], F32)
            cout2 = dp.tile([32 * CORES, NPC], F32, addr_space="Shared")
            cin3 = dp.tile([NPC, 128], F16)
            cout3 = dp.tile([N, 128], F16, addr_space="Shared")

            x4_local = pp.tile([128, WINS, 128], F16)

            for li, cfg in enumerate(LAYERS):
                if li >= STOP:
                    break
                cin, C, HC, elem = cfg['cin'], cfg['C'], cfg['HC'], cfg['elem']
                tdt = F16 if cfg['f16'] else F32
                htab = htabs[li]
                htab_writes = [[], []]
                with (
                    tc.tile_pool(name=f"hb{li}", bufs=3) as hb,
                    tc.tile_pool(name=f"hbp{li}", bufs=3, space="PSUM") as hbp,
                ):
                    waug_t = hb.tile([cin, HC + 4], F32, tag="w")
                    nc.sync.dma_start(waug_t[:], consts[f'waug{li}'][:, :])
                    baug_t = hb.tile([128, HC + 4], F32, tag="b")
                    nc.sync.dma_start(baug_t[:], consts[f'baug{li}'][:, :])
                    # h table build: h_aug = x @ Waug + baug
                    if li == 0:
                        blocks = [(xT1, 0, N)]
                    elif li == 1:
                        blocks = [(cout1, c * 16, NPC) for c in range(CORES)]
                    else:
                        blocks = [(cout2, c * 32, NPC) for c in range(CORES)]
                    row0 = 0
                    for (srcT, prow, ncols) in blocks:
                        for s0 in range(0, ncols, 512):
                            s1 = min(s0 + 512, ncols)
                            ww = s1 - s0
                            lhs = hb.tile([cin, 512], F32, tag="lhs")
                            nc.sync.dma_start(lhs[:, :ww], srcT[prow:prow + cin, s0:s1])
                            for j0 in range(0, ww, 128):
                                nn = min(128, ww - j0)
                                hp = hbp.tile([128, HC + 4], F32, space="PSUM", tag="hp")
                                nc.tensor.matmul(out=hp[:nn, :],
                                                 lhsT=lhs[:, j0:j0 + nn],
                                                 rhs=waug_t[:], start=True, stop=True)
                                hs = hb.tile([128, HC + 4], tdt, tag="hs")
                                nc.vector.tensor_tensor(out=hs[:nn, :], in0=hp[:nn, :],
                                                        in1=baug_t[:nn, :],
                                                        op=mybir.AluOpType.add)
                                wi_ = nc.scalar.dma_start(htab[row0:row0 + nn, 0:HC + 4], hs[:nn, :])
                                if row0 < HALF:
                                    htab_writes[0].append(wi_)
                                if row0 + nn > HALF:
                                    htab_writes[1].append(wi_)
                                row0 += nn

                # a_dst for own dst range
                xT_src = [xT_own, cin1, cin2][li]
                with (
                    tc.tile_pool(name=f"ad{li}", bufs=2) as ad,
                    tc.tile_pool(name=f"adp{li}", bufs=2, space="PSUM") as adp,
                ):
                    wadst_t = ad.tile([cin, 4], F32, tag="w")
                    nc.sync.dma_start(wadst_t[:], consts[f'wadst{li}'][:, :])
                    badst_t = ad.tile([128, 4], F32, tag="b")
                    nc.sync.dma_start(badst_t[:], consts[f'badst{li}'][:, :])
                    adst_all = pp.tile([128, WINS, 4], F32, tag="adst")
                    for w in range(WINS):
                        j0 = w * 128
                        j1 = min(j0 + 128, NPC)
                        nn = j1 - j0
                        xad = ad.tile([cin, 128], F32, tag="xad")
                        nc.scalar.dma_start(xad[:, :nn], xT_src[:, j0:j1])
                        ap_ = adp.tile([128, 4], F32, space="PSUM", tag="ap")
                        nc.tensor.matmul(out=ap_[:nn, :], lhsT=xad[:, :nn],
                                         rhs=wadst_t[:], start=True, stop=True)
                        nc.vector.tensor_tensor(out=adst_all[:nn, w, :], in0=ap_[:nn, :],
                                                in1=badst_t[:nn, :],
                                                op=mybir.AluOpType.add)

                # edge phase
                acc = pp.tile([128, WINS, HC + 4], F32, tag="acc")
                with (
                    tc.tile_pool(name=f"eg{li}", bufs=3) as eg,
                    tc.tile_pool(name=f"egp{li}", bufs=2, space="PSUM") as egp,
                    tc.tile_pool(name=f"winp{li}", bufs=2, space="PSUM") as winp,
                ):
                    psum_w = None
                    for bi in range(NB):
                        dflat = eg.tile([1, B * 128], F32, tag="dflat")
                        nc.sync.dma_start(dflat[:], dstflat_d[bi:bi + 1, :])
                        prow = egp.tile([128, B * 128], F32, space="PSUM", tag="prow")
                        for hh in range(B * 128 // 512):
                            nc.tensor.matmul(out=prow[:, hh * 512:(hh + 1) * 512],
                                             lhsT=ones_t[:],
                                             rhs=dflat[:, hh * 512:(hh + 1) * 512],
                                             start=True, stop=True)
                        oht = eg.tile([128, B * 128], F32, tag="oht")
                        nc.vector.tensor_scalar(
                            out=oht[:], in0=prow[:], scalar1=iota_col[:],
                            scalar2=None, op0=mybir.AluOpType.is_equal)
                        oh = eg.tile([128, B * 128], tdt, tag="oh")
                        nc.vector.tensor_tensor(
                            out=oh[:], in0=iota_rep[:],
                            in1=dstcolT_t[:, bi * B:(bi + 1) * B]
                                .unsqueeze(-1).to_broadcast([128, B, 128]),
                            op=mybir.AluOpType.is_equal)
                        if bi % 2 == 0:
                            g2 = eg.tile([128, 2 * B, elem], tdt, tag="g")
                            gi_ = nc.gpsimd.dma_gather(
                                g2[:], htab[0:HALF, :] if sched[bi * B][0] == 0
                                else htab[HALF:N, :],
                                wrapped_t[:, bi * B * 8:(bi + 2) * B * 8],
                                2 * B * 128, 2 * B * 128, elem,
                                single_packet=False)
                            for wi_ in htab_writes[sched[bi * B][0]]:
                                add_dep_helper(gi_.ins, wi_.ins, sync=True,
                                               reason="gather reads h table")
                        g = g2[:, (bi % 2) * B:(bi % 2) * B + B, :]
                        adst_e = egp.tile([128, B, 4], F32, space="PSUM", tag="adst_e")
                        for t in range(B):
                            hf, w, first, last, ev = sched[bi * B + t]
                            nc.tensor.matmul(
                                out=adst_e[:, t, :],
                                lhsT=oht[:, t * 128:(t + 1) * 128],
                                rhs=adst_all[:, w, :], start=True, stop=True)
                        lg = eg.tile([128, B, 4], F32, tag="lg")
                        nc.vector.tensor_tensor(out=lg[:], in0=g[:, :, HC:HC + 4],
                                                in1=adst_e[:],
                                                op=mybir.AluOpType.add)
                        lr = eg.tile([128, B, 4], F32, tag="lr")
                        nc.vector.scalar_tensor_tensor(
                            out=lr[:], in0=lg[:], scalar=NEG_GAT, in1=lg[:],
                            op0=mybir.AluOpType.mult, op1=mybir.AluOpType.max)
                        wt = eg.tile([128, B, 4], tdt, tag="wt")
                        nc.scalar.activation(wt[:], lr[:],
                                             mybir.ActivationFunctionType.Exp)
                        rhs = eg.tile([128, B, HC + 4], tdt, tag="rhs")
                        nc.vector.tensor_tensor(
                            out=rhs[:, :, 0:HC].rearrange("p b (h c) -> p b h c", h=H),
                            in0=g[:, :, 0:HC].rearrange("p b (h c) -> p b h c", h=H),
                            in1=wt[:].unsqueeze(-1).to_broadcast([128, B, 4, C]),
                            op=mybir.AluOpType.mult)
                        nc.vector.tensor_copy(out=rhs[:, :, HC:HC + 4], in_=wt[:])
                        for t in range(B):
                            hf, w, first, last, ev = sched[bi * B + t]
                            if first:
                                psum_w = winp.tile([128, HC + 4], F32, space="PSUM",
                                                   tag="win")
                            nc.tensor.matmul(out=psum_w[:],
                                             lhsT=oh[:, t * 128:(t + 1) * 128],
                                             rhs=rhs[:, t, :],
                                             start=first, stop=last)
                            if last:
                                if ev == 'copy':
                                    nc.vector.tensor_copy(out=acc[:, w, :], in_=psum_w[:])
                                else:
                                    nc.vector.tensor_tensor(
                                        out=acc[:, w, :], in0=acc[:, w, :],
                                        in1=psum_w[:], op=mybir.AluOpType.add)

                # normalization + output
                with (
                    tc.tile_pool(name=f"no{li}", bufs=3) as no,
                    tc.tile_pool(name=f"nop{li}", bufs=2, space="PSUM") as nop,
                ):
                    cin_t = [cin1, cin2, None][li]
                    if li < 2:
                        ident = no.tile([128, 128], F32, tag="id")
                        # identity built from iota compare: ident[p,j] = (iota_row[j]==p)
                        nc.vector.tensor_scalar(
                            out=ident[:], in0=iota128[:], scalar1=iota_col[:],
                            scalar2=None, op0=mybir.AluOpType.is_equal)
                    for w in range(WINS):
                        nn = 128 if w < WINS - 1 else LASTW
                        rec = no.tile([128, 4], F32, tag="rec")
                        nc.vector.reciprocal(rec[:nn, :], acc[:nn, w, HC:HC + 4])
                        tmp = no.tile([128, HC], F32, tag="tmp")
                        nc.vector.tensor_tensor(
                            out=tmp[:nn, :].rearrange("p (h c) -> p h c", h=H),
                            in0=acc[:nn, w, 0:HC].rearrange("p (h c) -> p h c", h=H),
                            in1=rec[:nn, :].unsqueeze(-1).to_broadcast([nn, 4, C]),
                            op=mybir.AluOpType.mult)
                        red = no.tile([128, C], F32, tag="red")
                        nc.vector.tensor_reduce(
                            out=red[:nn, :],
                            in_=tmp[:nn, :].rearrange("p (h c) -> p c h", h=H),
                            axis=mybir.AxisListType.X, op=mybir.AluOpType.add)
                        if li < 2:
                            tp = nop.tile([C, 128], F32, space="PSUM", tag="tp")
                            nc.tensor.transpose(out=tp[:], in_=red[:], identity=ident[:])
                            tps = no.tile([C, 128], F32, tag="tps")
                            nc.vector.tensor_copy(out=tps[:, :nn], in_=tp[:, :nn])
                            nc.sync.dma_start(cin_t[:, w * 128:w * 128 + nn],
                                              tps[:, :nn])
                        else:
                            nc.vector.tensor_copy(out=x4_local[:nn, w, 0:64],
                                                  in_=red[:nn, :])

                # inter-layer collective
                if li == 0:
                    nc.gpsimd.collective_compute(
                        "AllGather", mybir.AluOpType.bypass,
                        replica_groups=[list(range(CORES))],
                        ins=[cin1[:].opt()], outs=[cout1[:].opt()])
                elif li == 1:
                    nc.gpsimd.collective_compute(
                        "AllGather", mybir.AluOpType.bypass,
                        replica_groups=[list(range(CORES))],
                        ins=[cin2[:].opt()], outs=[cout2[:].opt()])
                else:
                    for w in range(WINS):
                        nn = 128 if w < WINS - 1 else LASTW
                        nc.sync.dma_start(cin3[w * 128:w * 128 + nn, :],
                                          x4_local[:nn, w, :])
                    cc3 = nc.gpsimd.collective_compute(
                        "AllGather", mybir.AluOpType.bypass,
                        replica_groups=[list(range(CORES))],
                        ins=[cin3[:].opt()], outs=[cout3[:].opt()])

            # ---------------- MLP phase ----------------
            with (
                tc.tile_pool(name="mc", bufs=1) as mc,
                tc.tile_pool(name="mw", bufs=3) as mw,
                tc.tile_pool(name="mp", bufs=2, space="PSUM") as mp,
            ):
                w1s_t = mc.tile([64, 64], F16)
                nc.sync.dma_start(w1s_t[:], w1s_d[:, :])
                w1t_t = mc.tile([64, 64], F16)
                nc.sync.dma_start(w1t_t[:], w1t_d[:, :])
                w1e_t = mc.tile([10, 64], F32)
                nc.sync.dma_start(w1e_t[:], w1e_d[:, :])
                b1m_t = mc.tile([64, 1], F32)
                nc.sync.dma_start(b1m_t[:], b1m_d[:, :])
                w2m_t = mc.tile([64, 16], F32)
                nc.sync.dma_start(w2m_t[:], w2m_d[:, :])
                b2m_t = mc.tile([16, 1], F32)
                nc.sync.dma_start(b2m_t[:], b2m_d[:, :])
                w3m_t = mc.tile([16, 1], F32)
                nc.sync.dma_start(w3m_t[:], w3m_d[:, :])
                b3m_t = mc.tile([1, 1], F32)
                nc.sync.dma_start(b3m_t[:], b3m_d[:, :])
                swr_t = mc.tile([128, Tm * 8], I16, tag="swr")
                nc.sync.dma_start(swr_t[:], swr_d[:, :])
                twr_t = mc.tile([128, Tm * 8], I16, tag="twr")
                nc.sync.dma_start(twr_t[:], twr_d[:, :])

                NE = B * 128
                for bm in range(min(NBm if STOP >= 4 else 0, int(os.environ.get('KERNEL_MLP_BATCHES', '100000')))):
                    gidx = int(batch_group[bm])
                    sh, th = gidx // 2, gidx % 2
                    gs = mw.tile([128, 1, NE], F16, tag="gs")
                    gt = mw.tile([128, 1, NE], F16, tag="gt")
                    gsi = nc.gpsimd.dma_gather(
                        gs[:], cout3[sh * HALF:(sh + 1) * HALF, :],
                        swr_t[:, bm * B * 8:(bm + 1) * B * 8],
                        NE, NE, 128, transpose=True, single_packet=False)
                    add_dep_helper(gsi.ins, cc3.ins, sync=True,
                                   reason="mlp gather reads x4 allgather")
                    gti = nc.gpsimd.dma_gather(
                        gt[:], cout3[th * HALF:(th + 1) * HALF, :],
                        twr_t[:, bm * B * 8:(bm + 1) * B * 8],
                        NE, NE, 128, transpose=True, single_packet=False)
                    add_dep_helper(gti.ins, cc3.ins, sync=True,
                                   reason="mlp gather reads x4 allgather")
                    et = mw.tile([10, NE], F32, tag="et")
                    nc.sync.dma_start(et[:], eT_d[:, bm * NE:(bm + 1) * NE])
                    res = mw.tile([1, NE], F32, tag="res")
                    for hh in range(NE // 512):
                        sl = slice(hh * 512, (hh + 1) * 512)
                        o1 = mp.tile([64, 512], F32, space="PSUM", tag="o1")
                        nc.tensor.matmul(out=o1[:], lhsT=w1s_t[:],
                                         rhs=gs[0:64, 0, sl], start=True, stop=False)
                        nc.tensor.matmul(out=o1[:], lhsT=w1t_t[:],
                                         rhs=gt[0:64, 0, sl], start=False, stop=False)
                        nc.tensor.matmul(out=o1[:], lhsT=w1e_t[:],
                                         rhs=et[:, sl], start=False, stop=True)
                        o1b = mw.tile([64, 512], F32, tag="o1b")
                        nc.vector.tensor_scalar(
                            out=o1b[:], in0=o1[:], scalar1=b1m_t[:, 0:1],
                            scalar2=None, op0=mybir.AluOpType.add)
                        o1s = mw.tile([64, 512], F32, tag="o1s")
                        nc.vector.scalar_tensor_tensor(
                            out=o1s[:], in0=o1b[:], scalar=NEG_MLP, in1=o1b[:],
                            op0=mybir.AluOpType.mult, op1=mybir.AluOpType.max)
                        o2 = mp.tile([16, 512], F32, space="PSUM", tag="o2")
                        nc.tensor.matmul(out=o2[:], lhsT=w2m_t[:], rhs=o1s[:],
                                         start=True, stop=True)
                        o2b = mw.tile([16, 512], F32, tag="o2b")
                        nc.vector.tensor_scalar(
                            out=o2b[:], in0=o2[:], scalar1=b2m_t[:, 0:1],
                            scalar2=None, op0=mybir.AluOpType.add)
                        o2s = mw.tile([16, 512], F32, tag="o2s")
                        nc.vector.scalar_tensor_tensor(
                            out=o2s[:], in0=o2b[:], scalar=NEG_MLP, in1=o2b[:],
                            op0=mybir.AluOpType.mult, op1=mybir.AluOpType.max)
                        o3 = mp.tile([1, 512], F32, space="PSUM", tag="o3")
                        nc.tensor.matmul(out=o3[:], lhsT=w3m_t[:], rhs=o2s[:],
                                         start=True, stop=True)
                        nc.scalar.activation(res[:, sl], o3[:],
                                             mybir.ActivationFunctionType.Sigmoid,
                                             bias=b3m_t[:, 0:1], scale=1.0)
                    nc.sync.dma_start(out_d[bm:bm + 1, :], res[:])

    nc.compile()
    return nc


# ----------------------------------------------------------------------------
# entry point
# ----------------------------------------------------------------------------

_CACHE = {}
LAST_EXEC_NS = None
LAST_RUN_S = None


def kernel(x, e, edge_index, params):
    x = np.asarray(x, np.float32)
    e = np.asarray(e, np.float32)
    ei = np.asarray(edge_index).astype(np.int64)

    sched, NB, wrapped, dstcolT, dstflat = build_gat_schedule(ei)
    NBm, Tm, swr, twr, outpos, batch_group, mlp_pc = build_mlp_schedule(ei)
    folded = fold_params(params)

    key = (NB, NBm, tuple(s[0] * 1000 + s[1] for s in sched[::37]),
           tuple(batch_group.tolist()))
    if key not in _CACHE:
        _CACHE[key] = build_program(sched, NB, NBm, Tm, batch_group)
    nc = _CACHE[key]

    xT1 = np.ascontiguousarray(x.T)
    iota_rep = np.tile(np.arange(128, dtype=np.float16)[None, :], (128, B))
    iota128 = np.tile(np.arange(128, dtype=np.float32)[None, :], (128, 1))
    iota_col = np.arange(128, dtype=np.float32)[:, None].copy()
    ones = np.ones((1, 128), np.float32)

    in_maps = []
    for c in range(CORES):
        ids, _, _ = mlp_pc[c]
        eT_c = np.zeros((10, Tm * 128), np.float32)
        valid = outpos[c] >= 0
        eT_c[:, valid] = e[outpos[c][valid]].T
        m = {
            "xT1": xT1,
            "xT_own": np.ascontiguousarray(xT1[:, c * NPC:(c + 1) * NPC]),
            "wrapped": wrapped[c],
            "dstcolT": dstcolT[c],
            "dstflat": dstflat[c],
            "swr": swr[c], "twr": twr[c], "eT": eT_c,
            "iota_rep": iota_rep, "iota128": iota128,
            "iota_col": iota_col, "ones": ones,
        }
        for li in range(3):
            m[f"waug{li}"] = folded[f'waug{li}']
            m[f"baug{li}"] = np.tile(folded[f'baug{li}'][None, :], (128, 1)).copy()
            m[f"wadst{li}"] = folded[f'wadst{li}']
            m[f"badst{li}"] = np.tile(folded[f'badst{li}'][None, :], (128, 1)).copy()
        m.update({"w1s": folded['w1s'], "w1t": folded['w1t'], "w1e": folded['w1e'],
                  "b1m": folded['b1m'][:, None].copy(),
                  "w2m": folded['w2m'], "b2m": folded['b2m'][:, None].copy(),
                  "w3m": folded['w3m'], "b3m": folded['b3m'][:, None].copy()})
        in_maps.append(m)

    import os as _os, time as _time
    trace = _os.environ.get('KERNEL_TRACE', '0') == '1'
    global LAST_EXEC_NS, LAST_RUN_S
    t0 = _time.time()
    try:
        res = run_bass_kernel_spmd(nc, in_maps, core_ids=list(range(CORES)),
                                   trace=trace)
    except Exception:
        if not trace:
            raise
        res = run_bass_kernel_spmd(nc, in_maps, core_ids=list(range(CORES)))
    LAST_RUN_S = _time.time() - t0
    LAST_EXEC_NS = getattr(res, 'exec_time_ns', None)

    out = np.zeros((E,), np.float32)
    for c in range(CORES):
        flat = res.results[c]["out"].reshape(-1)
        valid = outpos[c] >= 0
        out[outpos[c][valid]] = flat[valid]
    return out[:, None].astype(np.float32)


if __name__ == "__main__":
    data = np.load('/root/problem/cache_inputs.npz')
    import pickle
    with open('/root/problem/cache_params.pkl', 'rb') as f:
        params = pickle.load(f)
    out = kernel(data['x'], data['e'], data['edge_index'], params)
    np.save('/root/problem/kernel_out.npy', out)
    print("kernel out", out.shape, out[:5, 0])
